# revision 38
# baseline (speedup 1.0000x reference)
"""MoE routing + expert FFN kernel for 8 Trainium2 NeuronCores.

Sharding: data-parallel routing (core g owns token group g) + expert-parallel
FFN (core e owns expert e); dispatch/combine are on-device AllToAlls.

Per-core pipeline (SPMD, core id = g = e):
  1. Router: logitsT accumulated for all 8 token tiles in one [P, NT*E]
     PSUM tile (fp32 on PE), then one batched DVE/Act chain (softmax-max
     gate, argmax mask via broadcast compare, iota-sum expert index).
  2. Positions: per-tile cumsum via one triangular matmul + rank-1
     broadcast matmul for cross-tile offsets (7 tiny DVE prefix adds).
  3. Dispatch: ONE indirect-DMA row scatter (1024 token rows -> slot
     addresses, dropped tokens land on a dump row), then a single
     AllToAll over the full [T, H] slot buffer.
  4. Expert FFN (bf16, fp32 accum): hT = relu(w1.T @ x).T in SBUF,
     y = hT.T @ w2 in four H-quarters; each quarter's AllToAll #2 +
     indirect-gather combine overlaps the next quarter's matmuls.
"""

import sys

sys.path.insert(0, "/opt/trn_rl_repo")

import numpy as np
import ml_dtypes

G, T, H, E, DFF, CAP = 8, 1024, 1024, 8, 4096, 128
NCORES = 8
P = 128
NT = T // P   # 8 token tiles per group
NQ = 4        # H-quarters for M2/combine
HQ = H // NQ  # 256

_STATE = {}


def _build_nc(fake_collectives=False, stages=None):
    from concourse import bacc
    import concourse.bass as bass
    import concourse.mybir as mybir
    import concourse.tile as tile

    from concourse import library_config

    f32 = mybir.dt.float32
    bf16 = mybir.dt.bfloat16
    i32 = mybir.dt.int32
    i16 = mybir.dt.int16
    X = mybir.AxisListType.X
    AF = mybir.ActivationFunctionType
    OP = mybir.AluOpType

    nc = bacc.Bacc("TRN2", target_bir_lowering=False, debug=False,
                   num_devices=NCORES)

    tok_t = nc.dram_tensor("tok_t", [H, T], f32, kind="ExternalInput")
    tok_bf = nc.dram_tensor("tok_bf", [T, H], bf16, kind="ExternalInput")
    wr = nc.dram_tensor("wr", [H, E], f32, kind="ExternalInput")
    w1 = nc.dram_tensor("w1", [H, DFF], bf16, kind="ExternalInput")
    w2 = nc.dram_tensor("w2", [DFF, H], bf16, kind="ExternalInput")
    utri_c = nc.dram_tensor("utri_c", [P, P], bf16, kind="ExternalInput")
    ones_c = nc.dram_tensor("ones_c", [P, P], bf16, kind="ExternalInput")
    iota64 = nc.dram_tensor("iota64", [P, NT * E], f32, kind="ExternalInput")
    ones16 = nc.dram_tensor("ones16", [16, P], f32, kind="ExternalInput")
    out = nc.dram_tensor("out", [T, H], f32, kind="ExternalOutput")

    # Internal DRAM. xdisp has a dump row (index T) that dropped tokens
    # scatter into; ycomb's dump row is zeroed so dropped tokens gather 0.
    xdisp = nc.dram_tensor("xdisp", [T + 1, H], bf16)
    xrecv = nc.dram_tensor("xrecv", [T, H], bf16)
    addr_d = nc.dram_tensor("addr_d", [T, 1], i16)
    # M2/combine column chunks; smaller final chunks shrink the end-of-
    # kernel combine tail (which is pure latency after the last matmul).
    CHUNKS = [(0, 256), (256, 256), (512, 256), (768, 128), (896, 128)]
    yy = [nc.dram_tensor(f"yy{q}", [T, w], bf16)
          for q, (_, w) in enumerate(CHUNKS)]
    ycomb = [nc.dram_tensor(f"ycomb{q}", [T + 1, w], bf16)
             for q, (_, w) in enumerate(CHUNKS)]

    RG = [list(range(NCORES))]
    ALL = {"router", "cumsum", "dispatch", "transpose", "m1", "m2", "combine"}
    stg = ALL if stages is None else set(stages)

    def _a2a(out_ap, in_ap):
        if fake_collectives:
            nc.gpsimd.dma_start(out=out_ap, in_=in_ap)
        else:
            nc.gpsimd.collective_compute(
                "AllToAll", mybir.AluOpType.bypass, replica_groups=RG,
                ins=[in_ap.opt()], outs=[out_ap.opt()])

    with tile.TileContext(nc) as tc:
        nc.gpsimd.load_library(library_config.attnmlp)
        with (
            tc.tile_pool(name="const", bufs=1) as constp,
            tc.tile_pool(name="rt", bufs=1) as rtp,
            tc.tile_pool(name="big", bufs=1) as big,
            tc.tile_pool(name="w1s_p", bufs=2) as w1p,
            tc.tile_pool(name="psr", bufs=1, space="PSUM") as psr,
            tc.tile_pool(name="ps1", bufs=2, space="PSUM") as ps1,
            tc.tile_pool(name="psm", bufs=1, space="PSUM") as psm,
        ):
            # long-lived routing outputs (used by dispatch AND combine).
            # addr_pack holds the token->slot addresses in the packed
            # [16-partition-wrapped, replicated] int16 layout the gpsimd
            # dma_gather/dma_scatter_add ucode consumes.
            addr_pack = rtp.tile([P, T // 16], i16)
            scale_all = rtp.tile([P, NT], f32)
            ones16_sb = constp.tile([16, P], f32)
            ztile = big.tile([P, H], bf16)
            nc.vector.memset(ztile[:], 0.0)

            with tc.tile_pool(name="stage0", bufs=1) as st0:
                # ---- stage tokens FIRST (the critical path): transposed
                # f32 for the router (2 DMAs so router matmuls start after
                # the first half), bf16 row-major for the dispatch scatter.
                # DMA_ENGINES arbitration is request-order, so issue in the
                # order transfers are needed: tok_t half 1, the small consts
                # the routing chain needs, tok_t half 2, tokb, then w1 mb0.
                tokT_sb = st0.tile([P, NT * T], f32)
                tokb_sb = st0.tile([P, NT * H], bf16)
                nc.sync.dma_start(
                    tokT_sb[:, 0:4 * T].rearrange("p (k t) -> p k t", k=4),
                    tok_t[0:512, :].rearrange("(k p) t -> p k t", p=P))
                wr_sb = constp.tile([P, NT * E], f32)
                nc.sync.dma_start(wr_sb[:].rearrange("p (k e) -> p k e", e=E),
                                  wr[:, :].rearrange("(k p) e -> p k e", p=P))
                utri_sb = constp.tile([P, P], bf16)
                nc.sync.dma_start(utri_sb[:], utri_c[:, :])
                ones_sb = constp.tile([P, P], bf16)
                nc.sync.dma_start(ones_sb[:], ones_c[:, :])
                iota_sb = constp.tile([P, NT * E], f32)
                nc.sync.dma_start(iota_sb[:], iota64[:, :])
                nc.sync.dma_start(
                    tokT_sb[:, 4 * T:8 * T].rearrange("p (k t) -> p k t", k=4),
                    tok_t[512:1024, :].rearrange("(k p) t -> p k t", p=P))
                w1s0 = w1p.tile([P, 8 * 512], bf16)
                nc.sync.dma_start(
                    w1s0[:].rearrange("p (k f) -> p k f", k=8),
                    w1[:, 0:512].rearrange("(k p) f -> p k f", p=P))
                nc.sync.dma_start(
                    tokb_sb[:].rearrange("p (m h) -> p m h", m=NT),
                    tok_bf[:, :].rearrange("(m p) h -> p m h", p=P))
                nc.sync.dma_start(ones16_sb[:], ones16[:, :])
                zrow = constp.tile([1, HQ], bf16)
                nc.vector.memset(zrow[:], 0.0)
                for q, (_, w) in enumerate(CHUNKS):
                    nc.sync.dma_start(ycomb[q][T:T + 1, :], zrow[:, 0:w])

                # ---- router logits, two PSUM tiles (one per tok_t half so
                # matmuls start after the first token DMA; PSUM allows one
                # pending accumulation group per bank, so m is the outer loop)
                lgA = psr.tile([P, NT * E], f32, name="lgA", tag="r0")
                lgB = psr.tile([P, NT * E], f32, name="lgB", tag="r1")
                if "router" in stg:
                    for half, ps in ((0, lgA), (1, lgB)):
                        for m in range(NT):
                            for k4 in range(4):
                                k = half * 4 + k4
                                nc.tensor.matmul(
                                    ps[:, m * E:(m + 1) * E],
                                    lhsT=tokT_sb[:, k * T + m * P:
                                                 k * T + (m + 1) * P],
                                    rhs=wr_sb[:, k * E:(k + 1) * E],
                                    start=(k4 == 0), stop=(k4 == 3))
                # batched softmax / argmax meta
                lg = rtp.tile([P, NT * E], f32)
                maskf = rtp.tile([P, NT * E], f32)
                maskb = rtp.tile([P, NT * E], bf16)
                gate = rtp.tile([P, NT], f32)
                idx = rtp.tile([P, NT], f32)
                if "router" in stg:
                    # (two ops: hardware tensor ops may read at most one
                    # PSUM operand)
                    nc.vector.tensor_copy(lg[:], lgA[:])
                    nc.vector.tensor_tensor(lg[:], lg[:], lgB[:], op=OP.add)
                    nrmax = rtp.tile([P, NT], f32)
                    nc.vector.tensor_reduce(
                        nrmax[:], lg[:].rearrange("p (m e) -> p m e", e=E),
                        axis=X, op=OP.max, negate=True)
                    # fused (lg - max >= 0) per tile keeps the critical DVE
                    # chain short; exp(lg - max) via per-partition Act bias
                    # is off the address critical path.
                    for m in range(NT):
                        nc.vector.tensor_scalar(
                            maskf[:, m * E:(m + 1) * E],
                            lg[:, m * E:(m + 1) * E],
                            nrmax[:, m:m + 1], 0.0,
                            op0=OP.add, op1=OP.is_ge)
                    nc.vector.tensor_copy(maskb[:], maskf[:])
                    ex = rtp.tile([P, NT * E], f32)
                    for m in range(NT):
                        nc.scalar.activation(ex[:, m * E:(m + 1) * E],
                                             lg[:, m * E:(m + 1) * E],
                                             AF.Exp, bias=nrmax[:, m:m + 1])
                    esum = rtp.tile([P, NT], f32)
                    nc.vector.reduce_sum(
                        esum[:], ex[:].rearrange("p (m e) -> p m e", e=E),
                        axis=X)
                    nc.vector.reciprocal(gate[:], esum[:])
                    iw = rtp.tile([P, NT * E], f32)
                    nc.vector.tensor_tensor(iw[:], maskf[:], iota_sb[:],
                                            op=OP.mult)
                    nc.vector.reduce_sum(
                        idx[:], iw[:].rearrange("p (m e) -> p m e", e=E),
                        axis=X)

                # ---- capacity positions: per-tile cumsum (one triangular
                # matmul) + cross-tile offsets (rank-1 broadcast matmul)
                addr_f = rtp.tile([P, NT], f32)
                if "cumsum" in stg:
                    # batched cumsum over tokens: tile m's slice gets
                    # utri^T @ mask_m (intra-tile prefix) plus ones^T @
                    # mask_k for every earlier tile k (cross-tile counts)
                    cum_ps = psr.tile([P, NT * E], f32, name="cum_ps",
                                      tag="r0")
                    for m in range(NT):
                        for k in range(m + 1):
                            nc.tensor.matmul(
                                cum_ps[:, m * E:(m + 1) * E],
                                lhsT=(utri_sb[:] if k == m else ones_sb[:]),
                                rhs=maskb[:, k * E:(k + 1) * E],
                                start=(k == 0), stop=(k == m))
                    cum = rtp.tile([P, NT * E], f32)
                    nc.vector.tensor_copy(cum[:], cum_ps[:])
                    mcum = rtp.tile([P, NT * E], f32)
                    nc.vector.tensor_tensor(mcum[:], cum[:], maskf[:],
                                            op=OP.mult)
                    pos = rtp.tile([P, NT], f32)
                    nc.vector.reduce_sum(
                        pos[:], mcum[:].rearrange("p (m e) -> p m e", e=E),
                        axis=X)
                    nc.vector.tensor_scalar_sub(pos[:], pos[:], 1.0)
                    kept = rtp.tile([P, NT], f32)
                    nc.vector.tensor_scalar(kept[:], pos[:], float(CAP), None,
                                            op0=OP.is_lt)
                    drop = rtp.tile([P, NT], f32)
                    nc.vector.tensor_scalar(drop[:], pos[:], float(CAP), None,
                                            op0=OP.is_ge)
                    nc.vector.tensor_scalar_mul(addr_f[:], idx[:], float(CAP))
                    nc.vector.tensor_tensor(addr_f[:], addr_f[:], pos[:],
                                            op=OP.add)
                    nc.vector.tensor_tensor(addr_f[:], addr_f[:], kept[:],
                                            op=OP.mult)
                    nc.vector.tensor_scalar_mul(drop[:], drop[:], float(T))
                    nc.vector.tensor_tensor(addr_f[:], addr_f[:], drop[:],
                                            op=OP.add)
                    nc.vector.tensor_scalar_max(addr_f[:], addr_f[:], 0.0)
                    nc.vector.tensor_scalar_min(addr_f[:], addr_f[:],
                                                float(T))
                    nc.vector.tensor_tensor(scale_all[:], gate[:], kept[:],
                                            op=OP.mult)
                    # pack addresses for the gpsimd ucode: token t at
                    # packed position (t%16, t//16), then replicate the 16
                    # rows to all 128 partitions (doubling SBUF DMAs).
                    addr_i16 = rtp.tile([P, NT], i16)
                    nc.vector.tensor_copy(addr_i16[:], addr_f[:])
                    nc.sync.dma_start(
                        addr_d[:, :].rearrange("(m p) one -> p m one", p=P),
                        addr_i16[:].unsqueeze(2))
                    ap16 = rtp.tile([16, T // 16], f32)
                    ap16i = rtp.tile([16, T // 16], i16)
                    nc.sync.dma_start(
                        ap16i[:].unsqueeze(2),
                        addr_d[:, :].rearrange("(c q) one -> q c one", q=16))
                    nc.vector.tensor_copy(ap16[:], ap16i[:])
                    pk_ps = psr.tile([P, T // 16], f32, name="pk_ps",
                                     tag="r1")
                    nc.tensor.matmul(pk_ps[:], lhsT=ones16_sb[:],
                                     rhs=ap16[:], start=True, stop=True)
                    nc.vector.tensor_copy(addr_pack[:], pk_ps[:])

                # ---- dispatch: one gpsimd scatter-add of all 1024 token
                # rows onto the (pre-zeroed) slot buffer; dropped tokens
                # land on the dump row.
                if "dispatch" in stg:
                    nc.gpsimd.dma_scatter_add(
                        xdisp[:, :],
                        tokb_sb[:].rearrange("p (m h) -> p m h", m=NT),
                        addr_pack[:, :], T, T, H)
            # stage0 SBUF (tokT/tokb) released here

            if "dispatch" in stg:
                _a2a(xrecv[:, :], xdisp[0:T, :])

            # ---- transpose received slot rows into [H-part, slot] layout
            # with one identity-index transposing gather
            xt_sb = big.tile([P, NT * T], bf16)
            if "transpose" in stg:
                for k in range(8):
                    nc.sync.dma_start_transpose(
                        xt_sb[:, k * T:(k + 1) * T],
                        xrecv[:, k * P:(k + 1) * P])
                # re-zero xdisp for the NEXT run, now that the AllToAll has
                # consumed it (scatter-add needs a zeroed destination; a
                # priming run covers the very first execution). The dummy
                # multiply-by-zero makes these DMA requests wait for the
                # last transpose, so they cannot cut ahead of it on the
                # (request-ordered) DMA engines.
                nc.vector.tensor_scalar_mul(ztile[0:1, 0:1],
                                            xt_sb[0:1, 7 * T:7 * T + 1], 0.0)
                for m in range(NT):
                    nc.sync.dma_start(xdisp[m * P:(m + 1) * P, :], ztile[:])

            # ---- M1: hT[dff, slot] = relu(w1.T @ x) in bf16
            ht_sb = big.tile([P, 32 * T], bf16)
            for mb in range(_n_cnt("m1", stg, 8)):
                if mb == 0:
                    w1s = w1s0
                else:
                    w1s = w1p.tile([P, 8 * 512], bf16)
                    # structural gate: a dummy copy from xt_sb makes this
                    # load's DMA request arrive after the transposes', so
                    # streamed weights can't cut ahead of the dispatch
                    # chain on the (request-order) DMA engines.
                    nc.vector.tensor_copy(w1s[0:1, 0:1], xt_sb[0:1, 7 * T:7 * T + 1])
                    nc.sync.dma_start(
                        w1s[:].rearrange("p (k f) -> p k f", k=8),
                        w1[:, mb * 512:(mb + 1) * 512].rearrange(
                            "(k p) f -> p k f", p=P))
                for m4 in range(4):
                    mm = mb * 4 + m4
                    for n in range(2):
                        hps = ps1.tile([P, 512], f32)
                        for k in range(8):
                            nc.tensor.matmul(
                                hps[:],
                                lhsT=w1s[:, k * 512 + m4 * P:
                                         k * 512 + (m4 + 1) * P],
                                rhs=xt_sb[:, k * T + n * 512:
                                          k * T + (n + 1) * 512],
                                start=(k == 0), stop=(k == 7))
                        nc.scalar.activation(
                            ht_sb[:, mm * T + n * 512: mm * T + (n + 1) * 512],
                            hps[:], AF.Relu)

            # ---- M2 in H-quarters; each quarter's AllToAll #2 + combine
            # overlaps the next quarter's matmuls.
            with (
                tc.tile_pool(name="w2s_p", bufs=2) as w2p,
                tc.tile_pool(name="iob", bufs=2) as iop,
            ):
                def _combine(item):
                    cq, coff, cw = item
                    cb = iop.tile([P, NT * cw], bf16, name="cb", tag="cb")
                    nc.gpsimd.dma_gather(
                        cb[:].rearrange("p (m h) -> p m h", m=NT),
                        ycomb[cq][:, :], addr_pack[:, :], T, T, cw)
                    oc = iop.tile([P, NT * cw], f32, name="oc", tag="oc")
                    for m in range(NT):
                        nc.vector.tensor_scalar_mul(
                            oc[:, m * cw:(m + 1) * cw],
                            cb[:, m * cw:(m + 1) * cw],
                            scale_all[:, m:m + 1])
                    nc.sync.dma_start(
                        out[:, coff:coff + cw].rearrange(
                            "(m p) h -> p m h", p=P),
                        oc[:].rearrange("p (m h) -> p m h", m=NT))

                pending = None
                for q, (off, w) in enumerate(
                        CHUNKS[:_n_cnt("m2", stg, len(CHUNKS))]):
                    w2s = w2p.tile([P, 32 * w], bf16, name="w2s", tag="w2s")
                    if q == 0:
                        nc.vector.tensor_copy(w2s[0:1, 0:1], xt_sb[0:1, 7 * T:7 * T + 1])
                    elif q == 1:
                        nc.vector.tensor_copy(w2s[0:1, 0:1], ht_sb[0:1, 0:1])
                    nc.sync.dma_start(
                        w2s[:].rearrange("p (k h) -> p k h", k=32),
                        w2[:, off:off + w].rearrange("(k p) h -> p k h", p=P))
                    ycopy = iop.tile([P, NT * w], bf16, name="ycopy",
                                     tag="yc")
                    for st in range(NT):
                        pss = psm.tile([P, w], f32, name="pss",
                                       tag=f"s{st % 4}")
                        for kk in range(32):
                            nc.tensor.matmul(
                                pss[:],
                                lhsT=ht_sb[:, kk * T + st * P:
                                           kk * T + (st + 1) * P],
                                rhs=w2s[:, kk * w:(kk + 1) * w],
                                start=(kk == 0), stop=(kk == 31))
                        nc.vector.tensor_copy(ycopy[:, st * w:(st + 1) * w],
                                              pss[:])
                    nc.sync.dma_start(
                        yy[q][:, :].rearrange("(s p) h -> p s h", p=P),
                        ycopy[:].rearrange("p (s h) -> p s h", s=NT))
                    if "combine" in stg:
                        _a2a(ycomb[q][0:T, :], yy[q][:, :])
                        # defer chunk q-1's gather until after chunk q's
                        # AllToAll is issued: the Pool queue is in-order, so
                        # a gather sitting at its head must not block the
                        # next AllToAll.
                        if pending is not None:
                            _combine(pending)
                        pending = (q, off, w)
                if "combine" in stg and pending is not None:
                    _combine(pending)

    nc.compile()
    return nc


def _n_cnt(stage, stg, n):
    return n if stage in stg else 0


def _build_and_jit():
    import jax
    from jax.sharding import Mesh, PartitionSpec
    from jax.experimental.shard_map import shard_map
    import concourse.mybir as mybir
    from concourse import bass2jax

    nc = _build_nc()

    # ---- persistent PJRT runner (adapted from bass2jax.run_bass_via_pjrt,
    # built once so repeat kernel() calls reuse the compiled executable)
    bass2jax.install_neuronx_cc_hook()
    import concourse.mybir as mb

    partition_name = (nc.partition_id_tensor.name
                      if nc.partition_id_tensor else None)
    in_names, out_names, out_avals, zero_outs = [], [], [], []
    for alloc in nc.m.functions[0].allocations:
        if not isinstance(alloc, mb.MemoryLocationSet):
            continue
        name = alloc.memorylocations[0].name
        if alloc.kind == "ExternalInput":
            if name != partition_name:
                in_names.append(name)
        elif alloc.kind == "ExternalOutput":
            shape = tuple(alloc.tensor_shape)
            dtype = mb.dt.np(alloc.dtype)
            out_names.append(name)
            out_avals.append(jax.core.ShapedArray(shape, dtype))
            zero_outs.append(np.zeros(shape, dtype))
    n_params = len(in_names)
    n_outs = len(out_avals)
    in_names_all = list(in_names) + list(out_names)
    if partition_name is not None:
        in_names_all.append(partition_name)

    def _body(*args):
        operands = list(args)
        if partition_name is not None:
            operands.append(bass2jax.partition_id_tensor())
        outs = bass2jax._bass_exec_p.bind(
            *operands,
            out_avals=tuple(out_avals),
            in_names=tuple(in_names_all),
            out_names=tuple(out_names),
            lowering_input_output_aliases=(),
            sim_require_finite=True,
            sim_require_nnan=True,
            nc=nc,
        )
        return tuple(outs)

    devices = jax.devices()[:NCORES]
    mesh = Mesh(np.asarray(devices), ("core",))
    in_specs = (PartitionSpec("core"),) * (n_params + n_outs)
    out_specs = (PartitionSpec("core"),) * n_outs
    donate = tuple(range(n_params, n_params + n_outs))
    sharded = jax.jit(
        shard_map(_body, mesh=mesh, in_specs=in_specs,
                  out_specs=out_specs, check_rep=False),
        donate_argnums=donate, keep_unused=True)

    _STATE.update(dict(
        nc=nc, sharded=sharded, in_names=in_names, out_names=out_names,
        out_avals=out_avals, zero_outs=zero_outs, mesh=mesh))
    return _STATE


def _runner():
    if "sharded" not in _STATE:
        _build_and_jit()
    return _STATE


def make_in_maps(token_inputs, w_router, w1, w2):
    """Per-core input dicts (host-side shard/layout/dtype prep only)."""
    bf = ml_dtypes.bfloat16
    utri_c = np.triu(np.ones((P, P), np.float32)).astype(bf)
    ones_c = np.ones((P, P), dtype=bf)
    iota64 = np.tile(np.arange(E, dtype=np.float32), (P, NT))
    # identity indices packed (t%16, t//16), replicated to 128 partitions
    sel16 = np.tile(np.eye(16, dtype=np.float32), (1, 8))
    in_maps = []
    for g in range(NCORES):
        in_maps.append({
            "tok_t": np.ascontiguousarray(token_inputs[g].T.astype(np.float32)),
            "tok_bf": np.ascontiguousarray(token_inputs[g]).astype(bf),
            "wr": np.ascontiguousarray(w_router.astype(np.float32)),
            "w1": np.ascontiguousarray(w1[g]).astype(bf),
            "w2": np.ascontiguousarray(w2[g]).astype(bf),
            "utri_c": utri_c,
            "ones_c": ones_c,
            "iota64": iota64,
            "ones16": sel16,
        })
    return in_maps


def run_in_maps(in_maps):
    st = _runner()
    concat_in = [
        np.concatenate([np.asarray(in_maps[c][name])
                        for c in range(NCORES)], axis=0)
        for name in st["in_names"]
    ]
    concat_zeros = [np.zeros((NCORES * z.shape[0], *z.shape[1:]), z.dtype)
                    for z in st["zero_outs"]]
    out_arrs = st["sharded"](*concat_in, *concat_zeros)
    res = []
    for c in range(NCORES):
        res.append({
            name: np.asarray(out_arrs[i]).reshape(
                NCORES, *st["out_avals"][i].shape)[c]
            for i, name in enumerate(st["out_names"])
        })
    return res


def kernel(token_inputs, w_router, w1, w2, expert_capacity):
    token_inputs = np.asarray(token_inputs)
    w_router = np.asarray(w_router)
    w1 = np.asarray(w1)
    w2 = np.asarray(w2)
    assert int(expert_capacity) == CAP
    assert token_inputs.shape == (G, T, H)
    in_maps = make_in_maps(token_inputs, w_router, w1, w2)
    try:
        if not _STATE.get("primed"):
            # priming run: the dispatch scatter-add requires a zeroed slot
            # buffer, which each run re-zeroes for the next; the first
            # execution after load starts from unknown DRAM contents.
            run_in_maps(in_maps)
            _STATE["primed"] = True
        res = run_in_maps(in_maps)
    except Exception:
        # fallback: stock SPMD runner (recompiles per call, but robust)
        from concourse import bass_utils
        nc = _STATE.get("nc") or _build_nc()
        bass_utils.run_bass_kernel_spmd(
            nc, in_maps, core_ids=list(range(NCORES)))
        res = bass_utils.run_bass_kernel_spmd(
            nc, in_maps, core_ids=list(range(NCORES))).results
    return np.stack([res[g]["out"] for g in range(NCORES)], axis=0)
